# revision 19
# baseline (speedup 1.0000x reference)
"""Trainium2 Bass kernel for nn_CNN_RNN_30416958390487.

Pipeline: window/transpose (host) -> conv1 (Toeplitz matmul) -> leakyrelu+pool
-> conv2 (Toeplitz matmul) -> leakyrelu+pool -> feats AllGather -> Gi = feats@w_ih.T
(precomputed, weights streamed once) -> 20-step GRU with tensor-parallel split of
the 18816 gate rows across 8 cores (per-step AllGather of h) -> fc2.

All matmuls run in bf16 with f32 PSUM accumulation (validated ~3e-3 rel err on host).
"""

import os
import sys
import types
import numpy as np
import ml_dtypes

BF16 = ml_dtypes.bfloat16

# ---- problem constants (hardcoded; kernel.py must be self-contained) ----
B = 8            # batch / utterances
FREQ = 257
T = 1345
NF, NS = 128, 64
W = 20           # num windows: (T - NF - 1)//NS + 1
HID = 6272
G3 = 3 * HID     # 18816
NCLS = 10
NCORES = 8
NTP = G3 // NCORES   # 2352 gate rows per core
HP = HID // NCORES   # 784 hidden units per core
KC = HID // 128      # 49 contraction chunks for w_hh
KCG = 56             # Gi contraction chunks of 112 (= 2 * 28)
RESIDENT = int(os.environ.get("KB_RESIDENT", "20"))  # w_hh chunks kept in SBUF

# GRU matmul column-group N ranges (bank-aligned, 4 PE column groups)
GRP_RANGES = [(0, 512, [(0, 512)]),
              (512, 512, [(512, 512)]),
              (1024, 512, [(1024, 512)]),
              (1536, 816, [(1536, 512), (2048, 304)])]


def _gate_rows(c):
    j = np.arange(HP) + c * HP
    return np.concatenate([j, HID + j, 2 * HID + j])


def _feat_perm():
    """perm[j] = original feat index for contraction index j.

    Contraction chunks kb = b3*28 + h (112 partitions each); partition p maps to
    q3 = b3*112 + p with q3 = oc2*14 + w3; original = oc2*392 + h*14 + w3."""
    perm = np.empty(HID, np.int64)
    for b3 in range(2):
        for h in range(28):
            kb = b3 * 28 + h
            p = np.arange(112)
            q3 = b3 * 112 + p
            oc2, w3 = q3 // 14, q3 % 14
            perm[kb * 112 + p] = oc2 * 392 + h * 14 + w3
    return perm


def _prep_core(c, inp, perm):
    """Host-side layout prep for core c. Returns dict[str, np.ndarray]."""
    x = inp["x"]
    loc = x[c, 1:257, :]                       # [256, 1345]
    xw = np.zeros((W, 128, 260), np.float32)
    for k in range(W):
        xw[k, :, 2:258] = loc[:, 64 * k:64 * k + 128].T
    w1 = inp["conv1_w"]                        # [8, 1, 9, 9]
    t2 = np.zeros((9, 128, 992), np.float32)
    for dy in range(9):
        for oc in range(8):
            for dx in range(9):
                wo = np.arange(max(0, 2 - dx), min(124, 130 - dx))
                t2[dy, wo + dx - 2, oc * 124 + wo] = w1[oc, 0, dy, dx]
    b1row = np.repeat(inp["conv1_b"], 124)[None, :]          # [1, 992]

    w2w = inp["conv2_w"]                       # [16, 8, 4, 4]
    w2t = np.zeros((4, 384, 672), np.float32)
    for dy in range(4):
        for oc in range(16):
            for ic in range(8):
                for dx in range(4):
                    w2 = np.arange(max(0, 2 - dx), min(42, 43 - dx))
                    wi = w2 + dx - 2
                    ok = wi < 41
                    w2v, wiv = w2[ok], wi[ok]
                    w2t[dy, ic * 41 + wiv, oc * 42 + w2v] = w2w[oc, ic, dy, dx]
    w2t = w2t.reshape(4, 3, 128, 672)
    b2row = np.repeat(inp["conv2_b"], 42)[None, :]           # [1, 672]

    gr = _gate_rows(c)
    wih = inp["w_ih"][np.ix_(gr, perm)].T      # [6272, 2352]
    wih = wih.reshape(KCG, 112, NTP)
    wih2 = np.stack([wih[:, :, 0:1176], wih[:, :, 1176:2352]], axis=0)  # [2,56,112,1176]
    gb = inp["b_ih"][gr].copy()
    gb[0:2 * HP] += inp["b_hh"][gr[0:2 * HP]]  # fold b_hh for r,z gates
    gbrow = gb[None, :]                        # [1, 2352]

    whh = inp["w_hh"][gr].T.reshape(KC, 128, NTP)            # [49, 128, 2352]
    bhhn = np.tile(inp["b_hh"][2 * HID + c * HP: 2 * HID + (c + 1) * HP], (8, 1))

    h0 = inp["h0"]
    h0t = h0.T.reshape(KC, 128, 8).transpose(1, 0, 2).reshape(128, KC * 8)
    h0c = h0[:, c * HP:(c + 1) * HP]

    w2c = inp["fc2_w"].T.reshape(KC, 128, NCLS)
    b2c = inp["fc2_b"][None, :]

    return {
        "xwin": xw.astype(BF16),
        "t2": t2.astype(BF16),
        "b1row": b1row.astype(BF16),
        "w2t": w2t.astype(BF16),
        "b2row": b2row.astype(BF16),
        "wih": wih2.astype(BF16),
        "gbrow": gbrow.astype(BF16),
        "whh": whh.astype(BF16),
        "bhhn": bhhn.astype(np.float32),
        "h0t": h0t.astype(BF16),
        "h0c": h0c.astype(np.float32),
        "w2c": w2c.astype(BF16),
        "b2c": b2c.astype(BF16),
    }


def prep_all(inputs):
    inp = {k: np.asarray(v) for k, v in inputs.items()}
    perm = _feat_perm()
    return [_prep_core(c, inp, perm) for c in range(NCORES)]


# --------------------------------------------------------------------------
# Bass program
# --------------------------------------------------------------------------

_CACHED = {}


def _build(debug=False):
    import concourse.bacc as bacc
    import concourse.mybir as mybir
    from concourse import tile, masks
    from contextlib import ExitStack

    f32, bf16 = mybir.dt.float32, mybir.dt.bfloat16
    AOT = mybir.AluOpType
    AF = mybir.ActivationFunctionType
    RG = [list(range(NCORES))]

    nc = bacc.Bacc("TRN2", target_bir_lowering=False, debug=False,
                   num_devices=NCORES)

    din = {}
    def inp(name, shape, dt):
        din[name] = nc.dram_tensor(name, shape, dt, kind="ExternalInput")
        return din[name]

    xwin = inp("xwin", [W, 128, 260], bf16)
    t2 = inp("t2", [9, 128, 992], bf16)
    b1row = inp("b1row", [1, 992], bf16)
    w2t = inp("w2t", [4, 3, 128, 672], bf16)
    b2row = inp("b2row", [1, 672], bf16)
    wih = inp("wih", [2, KCG, 112, 1176], bf16)
    gbrow = inp("gbrow", [1, NTP], bf16)
    whh = inp("whh", [KC, 128, NTP], bf16)
    bhhn = inp("bhhn", [8, HP], f32)
    h0t = inp("h0t", [128, KC * 8], bf16)
    h0c = inp("h0c", [8, HP], f32)
    w2c = inp("w2c", [KC, 128, NCLS], bf16)
    b2c = inp("b2c", [1, NCLS], bf16)

    samples = nc.dram_tensor("samples", [8, NCLS], f32, kind="ExternalOutput")
    dbg = {}
    if debug:
        for nm, shp, dt in [("dbg_fa", [112, 560], bf16),
                            ("dbg_fb", [112, 560], bf16),
                            ("dbg_c2a", [128, 88], bf16),
                            ("dbg_gh0", [8, NTP], f32),
                            ("dbg_h1", [8, HP], f32),
                            ("dbg_gi0", [8, NTP], f32)]:
            dbg[nm] = nc.dram_tensor(nm, shp, dt, kind="ExternalOutput")

    with tile.TileContext(nc) as tc, ExitStack() as ctx:
        const = ctx.enter_context(tc.tile_pool(name="const", bufs=1))
        ident = const.tile([128, 128], bf16, name="ident")
        masks.make_identity(nc, ident[:])
        ones = const.tile([1, 128], bf16, name="ones")
        nc.vector.memset(ones[:], 1.0)

        gbs = const.tile([1, NTP], bf16, name="gbs")
        nc.sync.dma_start(gbs[:], gbrow[:])
        bhhns = const.tile([8, HP], f32, name="bhhns")
        nc.sync.dma_start(bhhns[:], bhhn[:])
        w2cs = const.tile([128, KC * NCLS], bf16, name="w2cs")
        for kb in range(KC):
            nc.sync.dma_start(w2cs[:, kb * NCLS:(kb + 1) * NCLS], w2c[kb])
        b2cs = const.tile([1, NCLS], bf16, name="b2cs")
        nc.sync.dma_start(b2cs[:], b2c[:])

        # resident w_hh chunks (loads interleaved into the conv loop below)
        whhres = ctx.enter_context(tc.tile_pool(name="whhres", bufs=1))
        wres = whhres.tile([128, RESIDENT * NTP], bf16, name="wres")

        dramp = ctx.enter_context(tc.tile_pool(name="dram", bufs=1, space="DRAM"))

        # =================== conv phase ===================
        with tc.tile_pool(name="featp", bufs=1) as featp, \
             tc.tile_pool(name="convp", bufs=2) as convp, \
             tc.tile_pool(name="psA", bufs=1, space="PSUM") as psA, \
             tc.tile_pool(name="psB", bufs=2, space="PSUM") as psB:
            # per-core feats accumulators (live until the feats AllGather)
            fa = featp.tile([112, W * 28], bf16, name="fa")
            fb = featp.tile([112, W * 28], bf16, name="fb")
            # conv-phase-only constants
            t2s = featp.tile([128, 9 * 992], bf16, name="t2s")
            for dy in range(9):
                nc.sync.dma_start(t2s[:, dy * 992:(dy + 1) * 992], t2[dy])
            w2s = featp.tile([128, 12 * 672], bf16, name="w2s")
            for dy in range(4):
                for j in range(3):
                    i = dy * 3 + j
                    nc.sync.dma_start(w2s[:, i * 672:(i + 1) * 672], w2t[dy, j])
            b1s = featp.tile([1, 992], bf16, name="b1s")
            nc.sync.dma_start(b1s[:], b1row[:])
            b2s = featp.tile([1, 672], bf16, name="b2s")
            nc.sync.dma_start(b2s[:], b2row[:])
            res_loaded = 0
            for k in range(W):
                xw = convp.tile([128, 260], bf16, name=f"xw{k}", tag="xw", bufs=3)
                nc.sync.dma_start(xw[:], xwin[k])
                # stagger resident w_hh loads through the conv phase
                want = min(RESIDENT, ((k + 1) * RESIDENT + W - 1) // W)
                while res_loaded < want:
                    r = res_loaded
                    nc.sync.dma_start(wres[:, r * NTP:(r + 1) * NTP], whh[r])
                    res_loaded += 1

                c2 = []
                for j in range(3):
                    c2t = convp.tile([128, 88], bf16, name=f"c2_{k}_{j}",
                                     tag=f"c2{j}", bufs=2)
                    nc.vector.memset(c2t[:], 0.0)
                    c2.append(c2t)

                for th in range(2):
                    p1 = psA.tile([126, 992], f32, name=f"p1_{k}_{th}",
                                  tag="p1", bufs=1)
                    for dy in range(9):
                        lhs = xw[:, th * 126 + dy: th * 126 + dy + 126]
                        for n0, nn in [(0, 512), (512, 480)]:
                            nc.tensor.matmul(
                                p1[:, n0:n0 + nn], lhs,
                                t2s[:, dy * 992 + n0: dy * 992 + n0 + nn],
                                start=(dy == 0), stop=False)
                    for n0, nn in [(0, 512), (512, 480)]:
                        nc.tensor.matmul(p1[:, n0:n0 + nn], ones[0:1, 0:126],
                                         b1s[0:1, n0:n0 + nn],
                                         start=False, stop=True)
                    # PSUM -> SBUF on ACT (tensor_tensor cannot read 2 PSUM operands)
                    s1 = convp.tile([126, 992], f32, name=f"s1_{k}_{th}",
                                    tag="s1", bufs=2)
                    nc.scalar.copy(s1[:], p1[:])
                    # pool-w (3->1) then leaky relu; free layout (oc, wq)
                    s1r = s1[:].rearrange("p (oc wo) -> p oc wo", oc=8)
                    pw = convp.tile([126, 328], f32, name=f"pw_{k}_{th}",
                                    tag="pw", bufs=2)
                    pwr = pw[:].rearrange("p (oc wq) -> p oc wq", oc=8)
                    nc.vector.tensor_max(pwr, s1r[:, :, 0:121:3], s1r[:, :, 1:122:3])
                    pw2 = convp.tile([126, 328], f32, name=f"pw2_{k}_{th}",
                                     tag="pw", bufs=2)
                    pw2r = pw2[:].rearrange("p (oc wq) -> p oc wq", oc=8)
                    nc.vector.tensor_max(pw2r, pwr, s1r[:, :, 2:123:3])
                    p1s = convp.tile([126, 328], bf16, name=f"p1s_{k}_{th}",
                                     tag="p1s", bufs=2)
                    nc.vector.scalar_tensor_tensor(
                        p1s[:], pw2[:], 0.01, pw2[:], op0=AOT.mult, op1=AOT.max)
                    # transpose 3 chunks; pool-h into c2 (with 2-col pad offset)
                    for j, (q0, qn) in enumerate([(0, 128), (128, 128), (256, 72)]):
                        tps = psB.tile([128, 126], bf16, name=f"tp_{k}_{th}_{j}",
                                       tag="tp", bufs=2)
                        nc.tensor.transpose(tps[0:qn, 0:126], p1s[0:126, q0:q0 + qn],
                                            ident[0:126, 0:126])
                        ts = convp.tile([128, 126], bf16, name=f"ts_{k}_{th}_{j}",
                                        tag="ts", bufs=2)
                        nc.scalar.copy(ts[0:qn, :], tps[0:qn, :])
                        tsr = ts[0:qn, :].rearrange("p (g r) -> p g r", r=3)
                        ph = convp.tile([128, 42], f32, name=f"ph_{k}_{th}_{j}",
                                        tag="ph", bufs=2)
                        nc.vector.tensor_max(ph[0:qn, :], tsr[:, :, 0], tsr[:, :, 1])
                        nc.vector.tensor_max(
                            c2[j][0:qn, 2 + th * 42: 44 + th * 42],
                            ph[0:qn, :], tsr[:, :, 2])

                if debug and k == 0:
                    nc.sync.dma_start(dbg["dbg_c2a"][:], c2[0][:])

                # conv2 (Toeplitz over w, K = (ic,w) in 3 chunks x 4 dy)
                p2 = psA.tile([85, 672], f32, name=f"p2_{k}", tag="p2", bufs=1)
                for dy in range(4):
                    for j in range(3):
                        i = dy * 3 + j
                        lhs = c2[j][:, dy:dy + 85]
                        for n0, nn in [(0, 512), (512, 160)]:
                            nc.tensor.matmul(
                                p2[:, n0:n0 + nn], lhs,
                                w2s[:, i * 672 + n0: i * 672 + n0 + nn],
                                start=(dy == 0 and j == 0), stop=False)
                for n0, nn in [(0, 512), (512, 160)]:
                    nc.tensor.matmul(p2[:, n0:n0 + nn], ones[0:1, 0:85],
                                     b2s[0:1, n0:n0 + nn], start=False, stop=True)
                s2 = convp.tile([85, 672], f32, name=f"s2_{k}", tag="s2", bufs=2)
                nc.scalar.copy(s2[:], p2[:])
                s2r = s2[:].rearrange("p (oc w) -> p oc w", oc=16)
                f0 = convp.tile([85, 224], f32, name=f"f0_{k}", tag="f0", bufs=2)
                f0r = f0[:].rearrange("p (oc w) -> p oc w", oc=16)
                nc.vector.tensor_max(f0r, s2r[:, :, 0:40:3], s2r[:, :, 1:41:3])
                f0b = convp.tile([85, 224], f32, name=f"f0b_{k}", tag="f0", bufs=2)
                f0br = f0b[:].rearrange("p (oc w) -> p oc w", oc=16)
                nc.vector.tensor_max(f0br, f0r, s2r[:, :, 2:42:3])
                f1 = convp.tile([85, 224], bf16, name=f"f1_{k}", tag="f1", bufs=2)
                nc.vector.scalar_tensor_tensor(
                    f1[:], f0b[:], 0.01, f0b[:], op0=AOT.mult, op1=AOT.max)
                for j in range(2):
                    tpf = psB.tile([112, 84], bf16, name=f"tpf_{k}_{j}",
                                   tag="tp", bufs=2)
                    nc.tensor.transpose(tpf[0:112, 0:84], f1[0:84, j * 112:(j + 1) * 112],
                                        ident[0:84, 0:84])
                    tsf = convp.tile([112, 84], bf16, name=f"tsf_{k}_{j}",
                                     tag="tsf", bufs=2)
                    nc.scalar.copy(tsf[:], tpf[:])
                    tfr = tsf[:].rearrange("p (g r) -> p g r", r=3)
                    pf = convp.tile([112, 28], f32, name=f"pf_{k}_{j}",
                                    tag="pf", bufs=2)
                    nc.vector.tensor_max(pf[:], tfr[:, :, 0], tfr[:, :, 1])
                    nc.vector.tensor_max((fa, fb)[j][:, k * 28:(k + 1) * 28],
                                         pf[:], tfr[:, :, 2])

            # ============ feats AllGather (inside featp scope) ============
            if debug:
                nc.sync.dma_start(dbg["dbg_fa"][:], fa[:])
                nc.sync.dma_start(dbg["dbg_fb"][:], fb[:])
            agfin = dramp.tile([224, W * 28], bf16, name="agfin")
            agfout = dramp.tile([224 * NCORES, W * 28], bf16, name="agfout",
                                addr_space="Shared")
            nc.sync.dma_start(agfin[0:112, :], fa[:])
            nc.sync.dma_start(agfin[112:224, :], fb[:])
            nc.gpsimd.collective_compute("AllGather", AOT.bypass, replica_groups=RG,
                                         ins=[agfin[:]], outs=[agfout[:]])

        # Gi tiles persist through the GRU
        gsp = ctx.enter_context(tc.tile_pool(name="gsp", bufs=1))
        gs = [gsp.tile([80, NTP], f32, name=f"gs{mt}") for mt in range(2)]

        # =================== Gi precompute ===================
        with tc.tile_pool(name="gip", bufs=1) as gip, \
             tc.tile_pool(name="psG", bufs=2, space="PSUM") as psG:
            # free layout (win, core, h): M index m = win*8 + core is single-stride
            fag = gip.tile([112, NCORES * W * 28], bf16, name="fag")
            fbg = gip.tile([112, NCORES * W * 28], bf16, name="fbg")
            agfr = agfout[:].rearrange("(c b3 p) (wl h) -> b3 c p wl h",
                                       c=NCORES, b3=2, p=112, h=28)
            for cc in range(NCORES):
                for b3, fg in enumerate((fag, fbg)):
                    dst = fg[:].rearrange("p (wl c h) -> p wl c h",
                                          c=NCORES, h=28)[:, :, cc, :]
                    nc.sync.dma_start(dst, agfr[b3, cc])
            for nh in range(2):
                gp = [psG.tile([80, 1176], f32, name=f"gp{nh}_{mt}", tag="gp", bufs=2)
                      for mt in range(2)]
                for kb in range(KCG):
                    b3, hh = kb // 28, kb % 28
                    wt = gip.tile([112, 1176], bf16, name=f"wt{nh}_{kb}",
                                  tag="wstr", bufs=3)
                    nc.sync.dma_start(wt[:], wih[nh, kb])
                    fg = (fag, fbg)[b3]
                    fgm = fg[:].rearrange("p (m h) -> p m h", h=28)
                    for mt in range(2):
                        lhs = fgm[:, mt * 80:(mt + 1) * 80, hh]
                        for n0, nn in [(0, 512), (512, 512), (1024, 152)]:
                            nc.tensor.matmul(gp[mt][:, n0:n0 + nn], lhs,
                                             wt[:, n0:n0 + nn],
                                             start=(kb == 0), stop=False)
                for mt in range(2):
                    for n0, nn in [(0, 512), (512, 512), (1024, 152)]:
                        nc.tensor.matmul(gp[mt][:, n0:n0 + nn], ones[0:1, 0:80],
                                         gbs[0:1, nh * 1176 + n0: nh * 1176 + n0 + nn],
                                         start=False, stop=True)
                    nc.vector.tensor_copy(gs[mt][:, nh * 1176:(nh + 1) * 1176],
                                          gp[mt][:])
        if debug:
            nc.sync.dma_start(dbg["dbg_gi0"][:], gs[0][0:8, :])

        # =================== GRU ===================
        with tc.tile_pool(name="grup", bufs=2) as grup, \
             tc.tile_pool(name="psR", bufs=1, space="PSUM") as psR:
            hT = grup.tile([128, KC * 8], bf16, name="hT_init", tag="hT", bufs=2)
            nc.sync.dma_start(hT[:], h0t[:])
            hcur = grup.tile([8, HP], f32, name="h_init", tag="h", bufs=2)
            nc.sync.dma_start(hcur[:], h0c[:])

            for t in range(W):
                G = psR.tile([128, NTP], f32, name=f"G{t}", tag="G", bufs=1)
                for kb in range(KC):
                    if kb < RESIDENT:
                        src = wres[:, kb * NTP:(kb + 1) * NTP]
                    else:
                        wst = grup.tile([128, NTP], bf16, name=f"wst{t}_{kb}",
                                        tag="wstr", bufs=3)
                        nc.sync.dma_start(wst[:], whh[kb])
                        src = wst[:]
                    lhs = hT[:, kb * 8:(kb + 1) * 8]
                    for g, (g0, gn, subs) in enumerate(GRP_RANGES):
                        for m0, mm in subs:
                            nc.tensor.matmul(G[32 * g:32 * g + 8, m0:m0 + mm],
                                             lhs, src[:, m0:m0 + mm],
                                             start=(kb == 0), stop=(kb == KC - 1),
                                             tile_position=(0, 32 * g))
                gh = grup.tile([8, NTP], f32, name=f"gh{t}", tag="gh", bufs=1)
                for g, (g0, gn, subs) in enumerate(GRP_RANGES):
                    nc.vector.tensor_copy(gh[:, g0:g0 + gn],
                                          G[32 * g:32 * g + 8, g0:g0 + gn])
                if debug and t == 0:
                    nc.sync.dma_start(dbg["dbg_gh0"][:], gh[:])

                mt, r0 = t // 10, (t % 10) * 8
                gi = grup.tile([8, NTP], f32, name=f"gi0_{t}", tag="gi0", bufs=1)
                nc.sync.dma_start(gi[:], gs[mt][r0:r0 + 8, :])
                prerz = grup.tile([8, 2 * HP], f32, name=f"prz{t}", tag="prz", bufs=1)
                nc.vector.tensor_add(prerz[:], gh[:, 0:2 * HP], gi[:, 0:2 * HP])
                rz = grup.tile([8, 2 * HP], f32, name=f"rz{t}", tag="rz", bufs=1)
                nc.scalar.activation(rz[:], prerz[:], AF.Sigmoid)
                t1 = grup.tile([8, HP], f32, name=f"t1_{t}", tag="gtmp", bufs=3)
                nc.vector.tensor_add(t1[:], gh[:, 2 * HP:3 * HP], bhhns[:])
                t2_ = grup.tile([8, HP], f32, name=f"t2_{t}", tag="gtmp", bufs=3)
                nc.vector.tensor_mul(t2_[:], rz[:, 0:HP], t1[:])
                t3 = grup.tile([8, HP], f32, name=f"t3_{t}", tag="gtmp", bufs=3)
                nc.vector.tensor_add(t3[:], t2_[:], gi[:, 2 * HP:3 * HP])
                nt = grup.tile([8, HP], f32, name=f"nt{t}", tag="nt", bufs=2)
                nc.scalar.activation(nt[:], t3[:], AF.Tanh)
                t4 = grup.tile([8, HP], f32, name=f"t4_{t}", tag="gtmp", bufs=3)
                nc.vector.tensor_sub(t4[:], hcur[:], nt[:])
                t5 = grup.tile([8, HP], f32, name=f"t5_{t}", tag="gtmp", bufs=3)
                nc.vector.tensor_mul(t5[:], rz[:, HP:2 * HP], t4[:])
                hnew = grup.tile([8, HP], f32, name=f"h{t + 1}", tag="h", bufs=2)
                nc.vector.tensor_add(hnew[:], nt[:], t5[:])
                if debug and t == 0:
                    nc.sync.dma_start(dbg["dbg_h1"][:], hnew[:])

                hb = grup.tile([8, HP], bf16, name=f"hb{t}", tag="hb", bufs=1)
                nc.vector.tensor_copy(hb[:], hnew[:])
                hTo = grup.tile([128, 56], bf16, name=f"hTo{t}", tag="hTo", bufs=2)
                for j in range(7):
                    pj = 128 if j < 6 else 16
                    pst = psR.tile([128, 8], bf16, name=f"pst{t}_{j}",
                                   tag="pst", bufs=2)
                    nc.tensor.transpose(pst[0:pj, 0:8], hb[0:8, j * 128:j * 128 + pj],
                                        ident[0:8, 0:8])
                    nc.vector.tensor_copy(hTo[0:pj, j * 8:(j + 1) * 8], pst[0:pj, 0:8])

                agin = dramp.tile([HP, 8], bf16, name=f"agin{t}", tag="agin", bufs=2)
                agout = dramp.tile([HID, 8], bf16, name=f"agout{t}", tag="agout",
                                   bufs=2, addr_space="Shared")
                nc.sync.dma_start(
                    agin[0:768, :].rearrange("(j p) c -> p j c", p=128),
                    hTo[:, 0:48].rearrange("p (j c) -> p j c", c=8))
                nc.sync.dma_start(agin[768:784, :], hTo[0:16, 48:56])
                nc.gpsimd.collective_compute("AllGather", AOT.bypass,
                                             replica_groups=RG,
                                             ins=[agin[:]], outs=[agout[:]])
                hT = grup.tile([128, KC * 8], bf16, name=f"hT{t + 1}", tag="hT",
                               bufs=2)
                nc.sync.dma_start(
                    hT[:].rearrange("p (kb c) -> p kb c", c=8),
                    agout[:].rearrange("(kb p) c -> p kb c", p=128))
                hcur = hnew

            # fc2 on the final gathered h
            ps2 = psR.tile([8, NCLS], f32, name="ps2", tag="ps2", bufs=1)
            for kb in range(KC):
                nc.tensor.matmul(ps2[:, :], hT[:, kb * 8:(kb + 1) * 8],
                                 w2cs[:, kb * NCLS:(kb + 1) * NCLS],
                                 start=(kb == 0), stop=False)
            nc.tensor.matmul(ps2[:, :], ones[0:1, 0:8], b2cs[0:1, :],
                             start=False, stop=True)
            souts = grup.tile([8, NCLS], f32, name="souts", tag="souts", bufs=1)
            nc.vector.tensor_copy(souts[:], ps2[:, :])
            nc.sync.dma_start(samples[:], souts[:])

    nc.compile()
    return nc


def _install_ntff_hook():
    if "antenv.axon_hooks" in sys.modules:
        return
    try:
        from trn_agent_boot.trn_boot import _ntff_profile_via_ctypes
        hook = _ntff_profile_via_ctypes("/opt/axon/libaxon_pjrt.so")
        mod = types.ModuleType("antenv.axon_hooks")
        mod.get_axon_ntff_profile_hook = lambda: hook
        sys.modules["antenv.axon_hooks"] = mod
    except Exception:
        pass


def get_program(debug=False):
    key = ("dbg" if debug else "std", RESIDENT)
    if key not in _CACHED:
        _CACHED[key] = _build(debug)
    return _CACHED[key]


def kernel(_trace=False, _debug=False, **inputs):
    from concourse.bass_utils import run_bass_kernel_spmd

    labels = np.asarray(inputs["labels"])
    preps = prep_all(inputs)
    nc = get_program(debug=_debug)
    if _trace:
        _install_ntff_hook()
    res = run_bass_kernel_spmd(nc, preps, core_ids=list(range(NCORES)),
                               trace=_trace)
    kernel.last_exec_time_ns = res.exec_time_ns
    kernel.last_results = res.results
    samples = np.asarray(res.results[0]["samples"], dtype=np.float32)
    return (samples, labels)


kernel.last_exec_time_ns = None
kernel.last_results = None
